# revision 50
# baseline (speedup 1.0000x reference)
"""Trainium2 Bass kernel for nn_BilinearAttention2 (gnn_message_passing).

Math (per graph g, head h — where "head" h is a raw C-order reshape of the
[nA, D] block into [H, nA, HD], i.e. head h = 16 consecutive nodes reshaped):
  x1 = A @ W1.T + b1 ; x2 = B @ W2.T + b2
  X1 = x1[g].flat[h*4096:(h+1)*4096].reshape(128, 32)   (likewise X2)
  att[i,j]  = sum_k tanh(X1[i,k] * X2[j,k]) * q[k]
  b2a = softmax_i(mean_j att); a2b = softmax_j(mean_i att)
  A_p[g,h] = X1.T @ b2a ; B_p[g,h] = X2.T @ a2b
  out[g] = concat(A_p[g].flat, B_p[g].flat)    -> [G, 2D]

Sharding: data-parallel over graphs. 8 cores x 2 graphs each; weights
replicated. Each core processes its 16 (g,h) pairs as 4 "stacks" of 4 pairs:
partition dim = (pair-in-stack, k) = (4, 32) = 128, free dim = (i', j') = 16384.

Engine allocation (per stack, all sized to hide under the ~14us ACT tanh):
  DVE   : broadcast-multiply at 2x packed mode; all b2a j-fold levels (2x
          contiguous-halves adds); a2b residual reduce; reciprocal.
  ACT   : tanh; softmax exp with fused accum_out sum; small PSUM->SBUF copies.
  PE    : a2b 32-chunk qdiag accumulate; b2a final qdiag matmul; transposes;
          projections.
  GPSIMD: compute-free (its tensor ops contend with DVE for the shared SBUF
          port, slowing both ~4x); used only as a DMA issue queue.
"""
import sys

sys.path.insert(0, "/opt/trn_rl_repo")

from contextlib import ExitStack

import numpy as np

import concourse.bass as bass
import concourse.bacc as bacc
import concourse.mybir as mybir
import concourse.tile as tile
from concourse.masks import make_identity

F32 = mybir.dt.float32
BF16 = mybir.dt.bfloat16

D = 256
H = 8
HD = 32
G = 16
NA = 128
NB = 128
NCORES = 8
GSH = G // NCORES          # graphs per core = 2
NPAIR = GSH * H            # 16 (g,h) pairs per core
SPP = 4                    # pairs per stack
NSTACK = NPAIR // SPP      # 4
NK = HD                    # 32
NJ = 128                   # nodes per head-view
FF = NJ * NJ               # 16384 free elems per stack


def build_kernel():
    nc = bacc.Bacc()
    a_d = nc.dram_tensor("A", [GSH * NA, D], F32, kind="ExternalInput")
    b_d = nc.dram_tensor("B", [GSH * NB, D], F32, kind="ExternalInput")
    w1_d = nc.dram_tensor("W1", [D, D], F32, kind="ExternalInput")
    w2_d = nc.dram_tensor("W2", [D, D], F32, kind="ExternalInput")
    b1_d = nc.dram_tensor("bias1", [1, D], F32, kind="ExternalInput")
    b2_d = nc.dram_tensor("bias2", [1, D], F32, kind="ExternalInput")
    q_d = nc.dram_tensor("q", [1, NK], F32, kind="ExternalInput")
    out_d = nc.dram_tensor("out", [GSH, 2 * D], F32, kind="ExternalOutput")

    # DRAM scratch
    x1_dram = nc.dram_tensor("x1_scratch", [GSH * NA, D], F32)
    x2_dram = nc.dram_tensor("x2_scratch", [GSH * NB, D], F32)

    with tile.TileContext(nc) as tc, ExitStack() as ctx:
        cst = ctx.enter_context(tc.tile_pool(name="cst", bufs=1))
        sbin = ctx.enter_context(tc.tile_pool(name="sbin", bufs=1))
        sbt = ctx.enter_context(tc.tile_pool(name="sbt", bufs=1))
        sbtr = ctx.enter_context(tc.tile_pool(name="sbtr", bufs=2))
        big = ctx.enter_context(tc.tile_pool(name="big", bufs=3))
        bigp = ctx.enter_context(tc.tile_pool(name="bigp", bufs=2))
        fold1 = ctx.enter_context(tc.tile_pool(name="fold1", bufs=2))
        fold2 = ctx.enter_context(tc.tile_pool(name="fold2", bufs=2))
        fold2b = ctx.enter_context(tc.tile_pool(name="fold2b", bufs=1))
        sm = ctx.enter_context(tc.tile_pool(name="sm", bufs=2))
        pst = ctx.enter_context(tc.tile_pool(name="pst", bufs=2, space="PSUM"))
        psx = ctx.enter_context(tc.tile_pool(name="psx", bufs=2, space="PSUM"))
        pss = ctx.enter_context(tc.tile_pool(name="pss", bufs=2, space="PSUM"))
        ps1 = ctx.enter_context(tc.tile_pool(name="ps1", bufs=1, space="PSUM"))

        ident = cst.tile([128, 128], F32)
        make_identity(nc, ident[:])
        ones1 = cst.tile([1, 128], F32)
        nc.vector.memset(ones1[:], 1.0)
        onescol = cst.tile([128, 1], F32)
        nc.vector.memset(onescol[:], 1.0)

        # ---- load inputs; transpose W1,W2 fully, A/B per row-block ----
        _ldq = [nc.sync, nc.scalar]

        def trans_rowblock(src_d, t_sb, r, name, qi=[0]):
            """transpose rows [128r, 128r+128) of src_d into t_sb[c][:, 128r:+128]"""
            blk = sbin.tile([128, D], F32, tag=f"{name}ld")
            _ldq[qi[0] % 2].dma_start(blk[:], src_d[r * 128:(r + 1) * 128, :])
            qi[0] += 1
            for c in range(2):
                tp = pst.tile([128, 128], F32, tag="tr")
                nc.tensor.transpose(tp[:], blk[:, c * 128:(c + 1) * 128], ident[:])
                nc.scalar.copy(t_sb[c][:, r * 128:(r + 1) * 128], tp[:])

        def alloc_t(name):
            return [sbin.tile([128, D], F32, tag=f"{name}T{c}", name=f"{name}T{c}") for c in range(2)]

        at, bt, w1t, w2t = alloc_t("A"), alloc_t("B"), alloc_t("W1"), alloc_t("W2")
        b1_sb = sbin.tile([1, D], F32)
        b2_sb = sbin.tile([1, D], F32)
        # g0-critical loads first, spread across queues
        trans_rowblock(b_d, bt, 0, "B")
        trans_rowblock(w2_d, w2t, 0, "W2")
        trans_rowblock(w2_d, w2t, 1, "W2")
        trans_rowblock(a_d, at, 0, "A")
        trans_rowblock(w1_d, w1t, 0, "W1")
        trans_rowblock(w1_d, w1t, 1, "W1")
        nc.sync.dma_start(b2_sb[:], b2_d[:])
        nc.scalar.dma_start(b1_sb[:], b1_d[:])

        _gq = [nc.sync]
        _gqi = [0]

        def _next_q():
            q = _gq[_gqi[0] % len(_gq)]
            _gqi[0] += 1
            return q

        def xmm(xt, wt, bb, xd, g, split=False):
            xp = psx.tile([128, D], F32, tag="xmm")
            nc.tensor.matmul(xp[:], xt[0][:, g * 128:(g + 1) * 128], wt[0][:], start=True, stop=False)
            nc.tensor.matmul(xp[:], xt[1][:, g * 128:(g + 1) * 128], wt[1][:], start=False, stop=False)
            nc.tensor.matmul(xp[:], ones1[0:1, :], bb[:], start=False, stop=True)
            xs = sbin.tile([128, D], F32, tag="xsb")
            if split:
                # hq=0 rows first so stack (g,0)'s gather can start early
                nc.scalar.copy(xs[0:64, :], xp[0:64, :])
                _next_q().dma_start(xd[g * 128:g * 128 + 64, :], xs[0:64, :])
                nc.scalar.copy(xs[64:128, :], xp[64:128, :])
                _next_q().dma_start(xd[g * 128 + 64:(g + 1) * 128, :], xs[64:128, :])
            else:
                nc.scalar.copy(xs[:], xp[:])
                nc.sync.dma_start(xd[g * 128:(g + 1) * 128, :], xs[:])

        x1f, x2f, x1t, x2t = [None] * NSTACK, [None] * NSTACK, [None] * NSTACK, [None] * NSTACK

        def gather_stack(s):
            g, hq = s // 2, s % 2
            for (xd, fl, tl, nm, quad) in ((x1_dram, x1f, x1t, "x1", True), (x2_dram, x2f, x2t, "x2", False)):
                xf = sbt.tile([128, 128], F32, tag=f"{nm}f{s}")
                srcv = xd[:].rearrange("(g2 hq pp n) (ss k) -> g2 hq n ss pp k", g2=GSH, hq=2, pp=SPP, ss=H)
                gq = nc.gpsimd if nm == "x1" else nc.sync
                gq.dma_start(xf[:], srcv[g, hq])
                fl[s] = xf
                tp = pst.tile([128, 128], F32, tag="tr")
                nc.tensor.transpose(tp[:], xf[:], ident[:])
                if quad:
                    xtb = sbtr.tile([128, 4 * 128], BF16, tag=f"{nm}tq")
                    nc.scalar.copy(xtb[:].rearrange("p (i q) -> p i q", q=4),
                                   tp[:].unsqueeze(2).broadcast_to([128, 128, 4]))
                else:
                    xtb = sbtr.tile([128, 128], BF16, tag=f"{nm}tp")
                    nc.scalar.copy(xtb[:], tp[:])
                tl[s] = xtb

        # ---- qdiag [128, SPP]: qdiag[(pp,k), pp'] = q[k] * (pp == pp') ----
        q_sb = cst.tile([1, NK], F32)
        nc.gpsimd.dma_start(q_sb[:], q_d[:])
        q_bf = cst.tile([1, NK], BF16)
        nc.vector.tensor_copy(q_bf[:], q_sb[:])
        qdiag = cst.tile([128, SPP], BF16)
        nc.vector.memset(qdiag[:], 0.0)
        for pp in range(SPP):
            nc.gpsimd.dma_start(qdiag[pp * NK:(pp + 1) * NK, pp:pp + 1], q_bf[:])

        # graph-0 chain first so stack 0 starts ASAP (x2 leads: it trails otherwise)
        xmm(bt, w2t, b2_sb, x2_dram, 0)
        xmm(at, w1t, b1_sb, x1_dram, 0)
        gather_stack(0)
        gather_stack(1)
        trans_rowblock(a_d, at, 1, "A")
        trans_rowblock(b_d, bt, 1, "B")
        xmm(bt, w2t, b2_sb, x2_dram, 1)
        xmm(at, w1t, b1_sb, x1_dram, 1)
        gather_stack(2)
        gather_stack(3)

        # ---- main loop over stacks, software-pipelined three deep:
        # main(s) -> epi_a(s) after main(s+1) -> epi_b(s) after main(s+2),
        # so per-stack epilogue latency never stalls the in-order engine
        # queues that feed the next stack's multiply/tanh stream.
        probs_at = sm.tile([128, NPAIR], F32, tag="pta")
        probs_bt = sm.tile([128, NPAIR], F32, tag="ptb")
        ADD = mybir.AluOpType.add
        a2b_ps_t, fs1_t = [None] * NSTACK, [None] * NSTACK

        def main_part(s):
            # t4 lives as two half-stack tiles (i 0:64 / 64:128) so the next
            # stack's tanh only back-pressures on the earliest-consumed half.
            t4h = [big.tile([128, FF // 2], BF16, tag="t4", name=f"t4_{s}_{h}")
                   for h in range(2)]
            fs1 = fold1.tile([128, 8192], BF16, tag="fs1")
            fs1_t[s] = fs1
            if s == 0:
                widths_i = (16, 16, 32, 64)
            elif s == NSTACK - 1:
                widths_i = (32, 32, 32, 32)
            else:
                widths_i = (64, 64)

            f1 = fs1[:].rearrange("p (i j) -> p i j", j=64)

            def t4v(i0, i1):
                """view [p, i0:i1, j] across the half-tiles (i0, i1 within one half)"""
                h, ib = i0 // 64, i0 % 64
                return t4h[h][:, ib * NJ:(ib + (i1 - i0)) * NJ]

            # b2a level-1 j-fold (128->64 within each i-row), issued per
            # completed tanh i-range: [p, i, 0:64] + [p, i, 64:128].
            def fold_l1(eng, i0, i1):
                tv = t4v(i0, i1).rearrange("p (i hj j) -> p i hj j", hj=2, j=64)
                eng.tensor_tensor(f1[:, i0:i1],
                                  tv[:, :, 0:1].squeeze(2),
                                  tv[:, :, 1:2].squeeze(2), op=ADD)

            io = 0
            for HH in widths_i:
                p4 = bigp.tile([128, FF // 2], BF16, tag="p4")
                in0 = x1t[s][:, io * 4:(io + HH) * 4].rearrange("p (i q) -> p i q", q=4)\
                    .unsqueeze(2).broadcast_to([128, HH, NJ // 4, 4])
                in1 = x2t[s][:].rearrange("p (j2 q) -> p j2 q", q=4)\
                    .unsqueeze(1).broadcast_to([128, HH, NJ // 4, 4])
                nc.vector.tensor_tensor(
                    p4[:, 0:HH * NJ].rearrange("p (i j2 q) -> p i j2 q", q=4, j2=NJ // 4),
                    in0, in1, op=mybir.AluOpType.mult)
                nc.scalar.activation(t4v(io, io + HH), p4[:, 0:HH * NJ],
                                     mybir.ActivationFunctionType.Tanh)
                io += HH
                if s == NSTACK - 1:
                    fold_l1(nc.vector, io - HH, io)   # tail stack: per tanh chunk
                elif io == 64:
                    fold_l1(nc.vector, 0, 64)
                elif io == 128:
                    fold_l1(nc.vector, 64, 128)

            # a2b: accumulate i'-quads on PE with q-block-diag lhsT
            a2b_ps = pss.tile([SPP, 4 * NJ], F32, tag="a2b")
            nch = FF // (4 * NJ)
            for ch in range(nch):
                half = t4h[ch // (nch // 2)]
                co = (ch % (nch // 2)) * 4 * NJ
                nc.tensor.matmul(a2b_ps[:], qdiag[:], half[:, co:co + 4 * NJ],
                                 start=(ch == 0), stop=(ch == nch - 1))
            a2b_ps_t[s] = a2b_ps

        fsl2_t = [None] * NSTACK

        def softmax_probs(s, lg, pt, nm):
            ex = sm.tile([SPP, NJ], F32, tag=f"ex{nm}")
            sme = sm.tile([SPP, 1], F32, tag=f"sm{nm}")
            nc.scalar.activation(ex[:], lg[:], mybir.ActivationFunctionType.Exp,
                                 scale=1.0 / NJ, accum_out=sme[:])
            rcp = sm.tile([SPP, 1], F32, tag=f"rc{nm}")
            nc.vector.reciprocal(rcp[:], sme[:])
            pr = sm.tile([SPP, NJ], F32, tag=f"pr{nm}")
            nc.scalar.mul(pr[:], ex[:], rcp[:])
            pp_ps = pst.tile([128, SPP], F32, tag="tr")
            nc.tensor.transpose(pp_ps[:], pr[:], ident[0:SPP, 0:SPP])
            nc.scalar.copy(pt[:, s * SPP:(s + 1) * SPP], pp_ps[:])

        _oq = [nc.sync, nc.gpsimd, nc.scalar]

        def project(s, xf, pt, half):
            g, hq = s // 2, s % 2
            pj = ps1.tile([NK, SPP], F32, tag="proj")
            for pp in range(SPP):
                nc.tensor.matmul(pj[:, pp:pp + 1], xf[:, pp * NK:(pp + 1) * NK],
                                 pt[:, s * SPP + pp:s * SPP + pp + 1], start=True, stop=True)
            pjs = sm.tile([NK, SPP], F32, tag=f"projs{s}_{half}")
            nc.scalar.copy(pjs[:], pj[:])
            dst = out_d[g:g + 1, half * D + hq * SPP * HD: half * D + (hq + 1) * SPP * HD]
            dst = dst.rearrange("o (pp k) -> o k pp", pp=SPP)
            _oq[(s * 2 + half) % 3].dma_start(dst, pjs[:])

        def epi_a(s):
            # a2b residual reduce + softmax + x2 projection; b2a fold level 2.
            a2b_t = sm.tile([SPP, NJ], F32, tag="a2bt")
            nc.vector.tensor_reduce(
                a2b_t[:], a2b_ps_t[s][:].rearrange("p (iq j) -> p j iq", iq=4),
                axis=mybir.AxisListType.X, op=mybir.AluOpType.add)
            softmax_probs(s, a2b_t, probs_bt, "b")
            project(s, x2f[s], probs_bt, 1)

            fsl2 = fold2.tile([128, 4096], BF16, tag="fsl2")
            fsl2_t[s] = fsl2
            vin = fs1_t[s][:].rearrange("p (i hj j) -> p i hj j", hj=2, j=32)
            nc.vector.tensor_tensor(fsl2[:].rearrange("p (i j) -> p i j", j=32),
                                    vin[:, :, 0:1].squeeze(2), vin[:, :, 1:2].squeeze(2), op=ADD)

        def epi_b(s):
            # b2a: finish the j-fold on DVE (levels 3..7), qdiag mm, softmax,
            # x1 projection.
            fs = fold2b.tile([128, 3968], BF16, tag="fs2b")
            src, win, off = fsl2_t[s], 32, 0
            base_in = 0
            while win > 1:
                w = win // 2
                vin = src[:, base_in:base_in + 128 * win].rearrange(
                    "p (i hj j) -> p i hj j", hj=2, j=w)
                nc.vector.tensor_tensor(
                    fs[:, off:off + 128 * w].rearrange("p (i j) -> p i j", j=w),
                    vin[:, :, 0:1].squeeze(2), vin[:, :, 1:2].squeeze(2), op=ADD)
                src, base_in, off, win = fs, off, off + 128 * w, w
            b2a_ps = ps1.tile([SPP, NJ], F32, tag="b2a")
            nc.tensor.matmul(b2a_ps[:], qdiag[:], fs[:, 3840:3968], start=True, stop=True)
            softmax_probs(s, b2a_ps, probs_at, "a")
            project(s, x1f[s], probs_at, 0)

        main_part(0)
        main_part(1)
        epi_a(0)
        main_part(2)
        epi_a(1)
        epi_b(0)
        main_part(3)
        epi_a(2)
        epi_b(1)
        epi_b(2)
        epi_a(3)
        epi_b(3)

    if not nc.is_finalized():
        nc.finalize()
    return nc


def shard_inputs(inputs):
    """Full inputs -> list of 8 per-core input maps."""
    A = np.asarray(inputs["A"], np.float32)
    B = np.asarray(inputs["B"], np.float32)
    maps = []
    for c in range(NCORES):
        maps.append({
            "A": np.ascontiguousarray(A[c * GSH * NA:(c + 1) * GSH * NA]),
            "B": np.ascontiguousarray(B[c * GSH * NB:(c + 1) * GSH * NB]),
            "W1": np.asarray(inputs["W1"], np.float32),
            "W2": np.asarray(inputs["W2"], np.float32),
            "bias1": np.asarray(inputs["bias1"], np.float32).reshape(1, D),
            "bias2": np.asarray(inputs["bias2"], np.float32).reshape(1, D),
            "q": np.asarray(inputs["q"], np.float32).reshape(1, NK),
        })
    return maps


_NC_CACHE = {}


def kernel(**inputs) -> np.ndarray:
    """Full (unsharded) inputs -> full [G, 2D] output, running on 8 cores."""
    from concourse.bass_utils import run_bass_kernel_spmd

    if "nc" not in _NC_CACHE:
        _NC_CACHE["nc"] = build_kernel()
    nc = _NC_CACHE["nc"]
    in_maps = shard_inputs(inputs)
    res = run_bass_kernel_spmd(nc, in_maps, core_ids=list(range(NCORES)))
    out = np.concatenate([res.results[c]["out"] for c in range(NCORES)], axis=0)
    return out.astype(np.float32)


if __name__ == "__main__":
    # CoreSim single-core debug: core 0 vs numpy reference
    from concourse.bass_interp import CoreSim

    rng = np.random.default_rng(0)
    scale = 1.0 / np.sqrt(D)
    full = {
        "A": rng.standard_normal((G * NA, D)).astype(np.float32),
        "B": rng.standard_normal((G * NB, D)).astype(np.float32),
        "W1": (rng.standard_normal((D, D)) * scale).astype(np.float32),
        "bias1": (rng.standard_normal(D) * scale).astype(np.float32),
        "W2": (rng.standard_normal((D, D)) * scale).astype(np.float32),
        "bias2": (rng.standard_normal(D) * scale).astype(np.float32),
        "q": (rng.standard_normal(HD) * scale).astype(np.float32),
    }

    def ref_core(m):
        x1 = m["A"] @ m["W1"].T + m["bias1"][0]
        x2 = m["B"] @ m["W2"].T + m["bias2"][0]
        x1 = x1.reshape(GSH, H, NA, HD)
        x2 = x2.reshape(GSH, H, NB, HD)
        att = np.einsum("ghijk,k->ghij", np.tanh(x1[:, :, :, None, :] * x2[:, :, None, :, :]), m["q"][0])

        def smax(v, ax):
            v = v - v.max(axis=ax, keepdims=True)
            e = np.exp(v)
            return e / e.sum(axis=ax, keepdims=True)

        b2a = smax(att.mean(axis=3), 2)
        a2b = smax(att.mean(axis=2), 2)
        A_p = np.einsum("ghik,ghi->ghk", x1, b2a).reshape(GSH, D)
        B_p = np.einsum("ghjk,ghj->ghk", x2, a2b).reshape(GSH, D)
        return np.concatenate([A_p, B_p], axis=1)

    nc = build_kernel()
    m0 = shard_inputs(full)[0]
    sim = CoreSim(nc)
    for k, v in m0.items():
        sim.tensor(k)[:] = v
    sim.simulate()
    got = sim.tensor("out").copy()
    want = ref_core(m0)
    err = np.abs(got - want).max() / np.abs(want).max()
    print("sim time:", sim.time, "ns")
    print("rel err:", err)



# revision 56
# speedup vs baseline: 1.2069x; 1.2069x over previous
"""Trainium2 Bass kernel for nn_BilinearAttention2 (gnn_message_passing).

Math (per graph g, head h — where "head" h is a raw C-order reshape of the
[nA, D] block into [H, nA, HD], i.e. head h = 16 consecutive nodes reshaped):
  x1 = A @ W1.T + b1 ; x2 = B @ W2.T + b2
  X1 = x1[g].flat[h*4096:(h+1)*4096].reshape(128, 32)   (likewise X2)
  att[i,j]  = sum_k tanh(X1[i,k] * X2[j,k]) * q[k]
  b2a = softmax_i(mean_j att); a2b = softmax_j(mean_i att)
  A_p[g,h] = X1.T @ b2a ; B_p[g,h] = X2.T @ a2b
  out[g] = concat(A_p[g].flat, B_p[g].flat)    -> [G, 2D]

Sharding: data-parallel over graphs. 8 cores x 2 graphs each; weights
replicated. Each core processes its 16 (g,h) pairs as 4 "stacks" of 4 pairs:
partition dim = (pair-in-stack, k) = (4, 32) = 128, free dim = (i', j') = 16384.

Engine allocation (per stack, all sized to hide under the ~14us ACT tanh):
  DVE   : broadcast-multiply at 2x packed mode; all b2a j-fold levels (2x
          contiguous-halves adds); a2b residual reduce; reciprocal.
  ACT   : tanh; softmax exp with fused accum_out sum; small PSUM->SBUF copies.
  PE    : a2b 32-chunk qdiag accumulate; b2a final qdiag matmul; transposes;
          projections.
  GPSIMD: compute-free (its tensor ops contend with DVE for the shared SBUF
          port, slowing both ~4x); used only as a DMA issue queue.
"""
import sys

sys.path.insert(0, "/opt/trn_rl_repo")

from contextlib import ExitStack

import numpy as np

import concourse.bass as bass
import concourse.bacc as bacc
import concourse.mybir as mybir
import concourse.tile as tile
from concourse.masks import make_identity

F32 = mybir.dt.float32
BF16 = mybir.dt.bfloat16

D = 256
H = 8
HD = 32
G = 16
NA = 128
NB = 128
NCORES = 8
GSH = G // NCORES          # graphs per core = 2
NPAIR = GSH * H            # 16 (g,h) pairs per core
SPP = 4                    # pairs per stack
NSTACK = NPAIR // SPP      # 4
NK = HD                    # 32
NJ = 128                   # nodes per head-view
FF = NJ * NJ               # 16384 free elems per stack


def build_kernel():
    nc = bacc.Bacc()
    a_d = nc.dram_tensor("A", [GSH * NA, D], F32, kind="ExternalInput")
    b_d = nc.dram_tensor("B", [GSH * NB, D], F32, kind="ExternalInput")
    w1_d = nc.dram_tensor("W1", [D, D], F32, kind="ExternalInput")
    w2_d = nc.dram_tensor("W2", [D, D], F32, kind="ExternalInput")
    b1_d = nc.dram_tensor("bias1", [1, D], F32, kind="ExternalInput")
    b2_d = nc.dram_tensor("bias2", [1, D], F32, kind="ExternalInput")
    q_d = nc.dram_tensor("q", [1, NK], F32, kind="ExternalInput")
    out_d = nc.dram_tensor("out", [GSH, 2 * D], F32, kind="ExternalOutput")


    with tile.TileContext(nc) as tc, ExitStack() as ctx:
        cst = ctx.enter_context(tc.tile_pool(name="cst", bufs=1))
        sbin = ctx.enter_context(tc.tile_pool(name="sbin", bufs=1))
        sbt = ctx.enter_context(tc.tile_pool(name="sbt", bufs=1))
        sbtr = ctx.enter_context(tc.tile_pool(name="sbtr", bufs=2))
        big = ctx.enter_context(tc.tile_pool(name="big", bufs=3))
        bigp = ctx.enter_context(tc.tile_pool(name="bigp", bufs=2))
        fold1 = ctx.enter_context(tc.tile_pool(name="fold1", bufs=2))
        fold2 = ctx.enter_context(tc.tile_pool(name="fold2", bufs=2))
        fold2b = ctx.enter_context(tc.tile_pool(name="fold2b", bufs=1))
        sm = ctx.enter_context(tc.tile_pool(name="sm", bufs=2))
        pst = ctx.enter_context(tc.tile_pool(name="pst", bufs=2, space="PSUM"))
        psx = ctx.enter_context(tc.tile_pool(name="psx", bufs=2, space="PSUM"))
        pss = ctx.enter_context(tc.tile_pool(name="pss", bufs=2, space="PSUM"))
        ps1 = ctx.enter_context(tc.tile_pool(name="ps1", bufs=1, space="PSUM"))

        ident = cst.tile([128, 128], F32)
        make_identity(nc, ident[:])
        ones1 = cst.tile([1, 128], F32)
        nc.vector.memset(ones1[:], 1.0)
        onescol = cst.tile([128, 1], F32)
        nc.vector.memset(onescol[:], 1.0)

        # ---- load inputs; transpose W1,W2 fully, A/B per row-block ----
        _ldq = [nc.sync, nc.scalar]

        def trans_rowblock(src_d, t_sb, r, name, qi=[0]):
            """transpose rows [128r, 128r+128) of src_d into t_sb[c][:, 128r:+128]"""
            blk = sbin.tile([128, D], F32, tag=f"{name}ld{r}")
            _ldq[qi[0] % 2].dma_start(blk[:], src_d[r * 128:(r + 1) * 128, :])
            qi[0] += 1
            for c in range(2):
                tp = pst.tile([128, 128], F32, tag="tr")
                nc.tensor.transpose(tp[:], blk[:, c * 128:(c + 1) * 128], ident[:])
                nc.scalar.copy(t_sb[c][:, r * 128:(r + 1) * 128], tp[:])

        def alloc_t(name):
            return [sbin.tile([128, D], F32, tag=f"{name}T{c}", name=f"{name}T{c}") for c in range(2)]

        at, bt, w1t, w2t = alloc_t("A"), alloc_t("B"), alloc_t("W1"), alloc_t("W2")
        b1_sb = sbin.tile([1, D], F32)
        b2_sb = sbin.tile([1, D], F32)
        # g0-critical loads first, spread across queues
        trans_rowblock(b_d, bt, 0, "B")
        trans_rowblock(w2_d, w2t, 0, "W2")
        trans_rowblock(w2_d, w2t, 1, "W2")
        trans_rowblock(a_d, at, 0, "A")
        trans_rowblock(w1_d, w1t, 0, "W1")
        trans_rowblock(w1_d, w1t, 1, "W1")
        nc.sync.dma_start(b2_sb[:], b2_d[:])
        nc.scalar.dma_start(b1_sb[:], b1_d[:])

        xs_t = {}

        def xmm(xt, wt, bb, nm, g):
            xp = psx.tile([128, D], F32, tag="xmm")
            nc.tensor.matmul(xp[:], xt[0][:, g * 128:(g + 1) * 128], wt[0][:], start=True, stop=False)
            nc.tensor.matmul(xp[:], xt[1][:, g * 128:(g + 1) * 128], wt[1][:], start=False, stop=False)
            nc.tensor.matmul(xp[:], ones1[0:1, :], bb[:], start=False, stop=True)
            xs = sbin.tile([128, D], F32, tag=f"xs_{nm}{g}", name=f"xs_{nm}{g}")
            nc.scalar.copy(xs[:], xp[:])
            xs_t[(nm, g)] = xs

        x1f, x2f, x1t, x2t = [None] * NSTACK, [None] * NSTACK, [None] * NSTACK, [None] * NSTACK

        def gather_stack(s):
            g, hq = s // 2, s % 2
            for (fl, tl, nm, quad) in ((x1f, x1t, "x1", True), (x2f, x2t, "x2", False)):
                xf = sbt.tile([128, 128], F32, tag=f"{nm}f{s}")
                # SBUF->SBUF permuting gather straight from the xmm output
                # (no DRAM round trip); 4 DMAs to stay within the 3-dim
                # balanced-AP limit. The memset only appeases CoreSim's
                # write tracking, which can't see partition-permuted writes.
                nc.vector.memset(xf[:], 0.0)
                xs = xs_t[(nm, g)]
                srcv = xs[:].rearrange("(hq pp n) (ss k) -> hq n ss pp k",
                                       hq=2, pp=SPP, n=16, ss=H)
                if s < 2:
                    # latency-critical first stacks: spread the 4 transfers
                    # over two queues each so they don't serialize
                    qs = ([nc.gpsimd, nc.scalar] if nm == "x1" else
                          [nc.sync, nc.scalar])
                else:
                    qs = [nc.gpsimd] if nm == "x1" else [nc.sync]
                for pp in range(SPP):
                    qs[pp % len(qs)].dma_start(xf[:, pp * NK:(pp + 1) * NK],
                                               srcv[hq][:, :, pp:pp + 1])
                fl[s] = xf
                tp = pst.tile([128, 128], F32, tag="tr")
                nc.tensor.transpose(tp[:], xf[:], ident[:])
                if quad:
                    xtb = sbtr.tile([128, 4 * 128], BF16, tag=f"{nm}tq")
                    nc.scalar.copy(xtb[:].rearrange("p (i q) -> p i q", q=4),
                                   tp[:].unsqueeze(2).broadcast_to([128, 128, 4]))
                else:
                    xtb = sbtr.tile([128, 128], BF16, tag=f"{nm}tp")
                    nc.scalar.copy(xtb[:], tp[:])
                tl[s] = xtb

        # ---- qdiag [128, SPP]: qdiag[(pp,k), pp'] = q[k] * (pp == pp') ----
        q_sb = cst.tile([1, NK], F32)
        nc.gpsimd.dma_start(q_sb[:], q_d[:])
        q_bf = cst.tile([1, NK], BF16)
        nc.vector.tensor_copy(q_bf[:], q_sb[:])
        qdiag = cst.tile([128, SPP], BF16)
        nc.vector.memset(qdiag[:], 0.0)
        for pp in range(SPP):
            nc.gpsimd.dma_start(qdiag[pp * NK:(pp + 1) * NK, pp:pp + 1], q_bf[:])

        # graph-0 chain first so stack 0 starts ASAP (x2 leads: it trails otherwise)
        xmm(bt, w2t, b2_sb, "x2", 0)
        xmm(at, w1t, b1_sb, "x1", 0)
        gather_stack(0)
        gather_stack(1)
        trans_rowblock(a_d, at, 1, "A")
        trans_rowblock(b_d, bt, 1, "B")
        xmm(bt, w2t, b2_sb, "x2", 1)
        xmm(at, w1t, b1_sb, "x1", 1)
        gather_stack(2)
        gather_stack(3)

        # ---- main loop over stacks, software-pipelined three deep:
        # main(s) -> epi_a(s) after main(s+1) -> epi_b(s) after main(s+2),
        # so per-stack epilogue latency never stalls the in-order engine
        # queues that feed the next stack's multiply/tanh stream.
        probs_at = sm.tile([128, NPAIR], F32, tag="pta")
        probs_bt = sm.tile([128, NPAIR], F32, tag="ptb")
        ADD = mybir.AluOpType.add
        a2b_ps_t, fs1_t = [None] * NSTACK, [None] * NSTACK

        def main_part(s):
            # t4 lives as two half-stack tiles (i 0:64 / 64:128) so the next
            # stack's tanh only back-pressures on the earliest-consumed half.
            t4h = [big.tile([128, FF // 2], BF16, tag="t4", name=f"t4_{s}_{h}")
                   for h in range(2)]
            fs1 = fold1.tile([128, 8192], BF16, tag="fs1")
            fs1_t[s] = fs1
            if s == 0:
                widths_i = (16, 16, 32, 64)
            elif s == NSTACK - 1:
                widths_i = (32, 32, 32, 32)
            else:
                widths_i = (64, 64)

            f1 = fs1[:].rearrange("p (i j) -> p i j", j=64)

            def t4v(i0, i1):
                """view [p, i0:i1, j] across the half-tiles (i0, i1 within one half)"""
                h, ib = i0 // 64, i0 % 64
                return t4h[h][:, ib * NJ:(ib + (i1 - i0)) * NJ]

            # b2a level-1 j-fold (128->64 within each i-row), issued per
            # completed tanh i-range: [p, i, 0:64] + [p, i, 64:128].
            def fold_l1(eng, i0, i1):
                tv = t4v(i0, i1).rearrange("p (i hj j) -> p i hj j", hj=2, j=64)
                eng.tensor_tensor(f1[:, i0:i1],
                                  tv[:, :, 0:1].squeeze(2),
                                  tv[:, :, 1:2].squeeze(2), op=ADD)

            io = 0
            for HH in widths_i:
                p4 = bigp.tile([128, FF // 2], BF16, tag="p4")
                in0 = x1t[s][:, io * 4:(io + HH) * 4].rearrange("p (i q) -> p i q", q=4)\
                    .unsqueeze(2).broadcast_to([128, HH, NJ // 4, 4])
                in1 = x2t[s][:].rearrange("p (j2 q) -> p j2 q", q=4)\
                    .unsqueeze(1).broadcast_to([128, HH, NJ // 4, 4])
                nc.vector.tensor_tensor(
                    p4[:, 0:HH * NJ].rearrange("p (i j2 q) -> p i j2 q", q=4, j2=NJ // 4),
                    in0, in1, op=mybir.AluOpType.mult)
                nc.scalar.activation(t4v(io, io + HH), p4[:, 0:HH * NJ],
                                     mybir.ActivationFunctionType.Tanh)
                io += HH
                if s == NSTACK - 1:
                    fold_l1(nc.vector, io - HH, io)   # tail stack: per tanh chunk
                elif io == 64:
                    fold_l1(nc.vector, 0, 64)
                elif io == 128:
                    fold_l1(nc.vector, 64, 128)

            # a2b: accumulate i'-quads on PE with q-block-diag lhsT
            a2b_ps = pss.tile([SPP, 4 * NJ], F32, tag="a2b")
            nch = FF // (4 * NJ)
            for ch in range(nch):
                half = t4h[ch // (nch // 2)]
                co = (ch % (nch // 2)) * 4 * NJ
                nc.tensor.matmul(a2b_ps[:], qdiag[:], half[:, co:co + 4 * NJ],
                                 start=(ch == 0), stop=(ch == nch - 1))
            a2b_ps_t[s] = a2b_ps

        fsl2_t = [None] * NSTACK

        def softmax_probs(s, lg, pt, nm):
            ex = sm.tile([SPP, NJ], F32, tag=f"ex{nm}")
            sme = sm.tile([SPP, 1], F32, tag=f"sm{nm}")
            nc.scalar.activation(ex[:], lg[:], mybir.ActivationFunctionType.Exp,
                                 scale=1.0 / NJ, accum_out=sme[:])
            rcp = sm.tile([SPP, 1], F32, tag=f"rc{nm}")
            nc.vector.reciprocal(rcp[:], sme[:])
            pr = sm.tile([SPP, NJ], F32, tag=f"pr{nm}")
            nc.scalar.mul(pr[:], ex[:], rcp[:])
            pp_ps = pst.tile([128, SPP], F32, tag="tr")
            nc.tensor.transpose(pp_ps[:], pr[:], ident[0:SPP, 0:SPP])
            nc.scalar.copy(pt[:, s * SPP:(s + 1) * SPP], pp_ps[:])

        _oq = [nc.sync, nc.gpsimd, nc.scalar]

        def project(s, xf, pt, half):
            g, hq = s // 2, s % 2
            pj = ps1.tile([NK, SPP], F32, tag="proj")
            for pp in range(SPP):
                nc.tensor.matmul(pj[:, pp:pp + 1], xf[:, pp * NK:(pp + 1) * NK],
                                 pt[:, s * SPP + pp:s * SPP + pp + 1], start=True, stop=True)
            pjs = sm.tile([NK, SPP], F32, tag=f"projs{s}_{half}")
            nc.scalar.copy(pjs[:], pj[:])
            dst = out_d[g:g + 1, half * D + hq * SPP * HD: half * D + (hq + 1) * SPP * HD]
            dst = dst.rearrange("o (pp k) -> o k pp", pp=SPP)
            _oq[(s * 2 + half) % 3].dma_start(dst, pjs[:])

        def epi_a(s):
            # a2b residual reduce + softmax + x2 projection; b2a fold level 2.
            a2b_t = sm.tile([SPP, NJ], F32, tag="a2bt")
            nc.vector.tensor_reduce(
                a2b_t[:], a2b_ps_t[s][:].rearrange("p (iq j) -> p j iq", iq=4),
                axis=mybir.AxisListType.X, op=mybir.AluOpType.add)
            softmax_probs(s, a2b_t, probs_bt, "b")
            project(s, x2f[s], probs_bt, 1)

            fsl2 = fold2.tile([128, 4096], BF16, tag="fsl2")
            fsl2_t[s] = fsl2
            vin = fs1_t[s][:].rearrange("p (i hj j) -> p i hj j", hj=2, j=32)
            nc.vector.tensor_tensor(fsl2[:].rearrange("p (i j) -> p i j", j=32),
                                    vin[:, :, 0:1].squeeze(2), vin[:, :, 1:2].squeeze(2), op=ADD)

        def epi_b(s):
            # b2a: finish the j-fold on DVE (levels 3..7), qdiag mm, softmax,
            # x1 projection.
            fs = fold2b.tile([128, 3968], BF16, tag="fs2b")
            src, win, off = fsl2_t[s], 32, 0
            base_in = 0
            while win > 1:
                w = win // 2
                vin = src[:, base_in:base_in + 128 * win].rearrange(
                    "p (i hj j) -> p i hj j", hj=2, j=w)
                nc.vector.tensor_tensor(
                    fs[:, off:off + 128 * w].rearrange("p (i j) -> p i j", j=w),
                    vin[:, :, 0:1].squeeze(2), vin[:, :, 1:2].squeeze(2), op=ADD)
                src, base_in, off, win = fs, off, off + 128 * w, w
            b2a_ps = ps1.tile([SPP, NJ], F32, tag="b2a")
            nc.tensor.matmul(b2a_ps[:], qdiag[:], fs[:, 3840:3968], start=True, stop=True)
            softmax_probs(s, b2a_ps, probs_at, "a")
            project(s, x1f[s], probs_at, 0)

        main_part(0)
        main_part(1)
        epi_a(0)
        main_part(2)
        epi_a(1)
        epi_b(0)
        main_part(3)
        epi_a(2)
        epi_b(1)
        epi_b(2)
        epi_a(3)
        epi_b(3)

    if not nc.is_finalized():
        nc.finalize()
    return nc


def shard_inputs(inputs):
    """Full inputs -> list of 8 per-core input maps."""
    A = np.asarray(inputs["A"], np.float32)
    B = np.asarray(inputs["B"], np.float32)
    maps = []
    for c in range(NCORES):
        maps.append({
            "A": np.ascontiguousarray(A[c * GSH * NA:(c + 1) * GSH * NA]),
            "B": np.ascontiguousarray(B[c * GSH * NB:(c + 1) * GSH * NB]),
            "W1": np.asarray(inputs["W1"], np.float32),
            "W2": np.asarray(inputs["W2"], np.float32),
            "bias1": np.asarray(inputs["bias1"], np.float32).reshape(1, D),
            "bias2": np.asarray(inputs["bias2"], np.float32).reshape(1, D),
            "q": np.asarray(inputs["q"], np.float32).reshape(1, NK),
        })
    return maps


_NC_CACHE = {}


def kernel(**inputs) -> np.ndarray:
    """Full (unsharded) inputs -> full [G, 2D] output, running on 8 cores."""
    from concourse.bass_utils import run_bass_kernel_spmd

    if "nc" not in _NC_CACHE:
        _NC_CACHE["nc"] = build_kernel()
    nc = _NC_CACHE["nc"]
    in_maps = shard_inputs(inputs)
    res = run_bass_kernel_spmd(nc, in_maps, core_ids=list(range(NCORES)))
    out = np.concatenate([res.results[c]["out"] for c in range(NCORES)], axis=0)
    return out.astype(np.float32)


if __name__ == "__main__":
    # CoreSim single-core debug: core 0 vs numpy reference
    from concourse.bass_interp import CoreSim

    rng = np.random.default_rng(0)
    scale = 1.0 / np.sqrt(D)
    full = {
        "A": rng.standard_normal((G * NA, D)).astype(np.float32),
        "B": rng.standard_normal((G * NB, D)).astype(np.float32),
        "W1": (rng.standard_normal((D, D)) * scale).astype(np.float32),
        "bias1": (rng.standard_normal(D) * scale).astype(np.float32),
        "W2": (rng.standard_normal((D, D)) * scale).astype(np.float32),
        "bias2": (rng.standard_normal(D) * scale).astype(np.float32),
        "q": (rng.standard_normal(HD) * scale).astype(np.float32),
    }

    def ref_core(m):
        x1 = m["A"] @ m["W1"].T + m["bias1"][0]
        x2 = m["B"] @ m["W2"].T + m["bias2"][0]
        x1 = x1.reshape(GSH, H, NA, HD)
        x2 = x2.reshape(GSH, H, NB, HD)
        att = np.einsum("ghijk,k->ghij", np.tanh(x1[:, :, :, None, :] * x2[:, :, None, :, :]), m["q"][0])

        def smax(v, ax):
            v = v - v.max(axis=ax, keepdims=True)
            e = np.exp(v)
            return e / e.sum(axis=ax, keepdims=True)

        b2a = smax(att.mean(axis=3), 2)
        a2b = smax(att.mean(axis=2), 2)
        A_p = np.einsum("ghik,ghi->ghk", x1, b2a).reshape(GSH, D)
        B_p = np.einsum("ghjk,ghj->ghk", x2, a2b).reshape(GSH, D)
        return np.concatenate([A_p, B_p], axis=1)

    nc = build_kernel()
    m0 = shard_inputs(full)[0]
    sim = CoreSim(nc)
    for k, v in m0.items():
        sim.tensor(k)[:] = v
    sim.simulate()
    got = sim.tensor("out").copy()
    want = ref_core(m0)
    err = np.abs(got - want).max() / np.abs(want).max()
    print("sim time:", sim.time, "ns")
    print("rel err:", err)



# revision 58
# speedup vs baseline: 1.2159x; 1.0075x over previous
"""Trainium2 Bass kernel for nn_BilinearAttention2 (gnn_message_passing).

Math (per graph g, head h — where "head" h is a raw C-order reshape of the
[nA, D] block into [H, nA, HD], i.e. head h = 16 consecutive nodes reshaped):
  x1 = A @ W1.T + b1 ; x2 = B @ W2.T + b2
  X1 = x1[g].flat[h*4096:(h+1)*4096].reshape(128, 32)   (likewise X2)
  att[i,j]  = sum_k tanh(X1[i,k] * X2[j,k]) * q[k]
  b2a = softmax_i(mean_j att); a2b = softmax_j(mean_i att)
  A_p[g,h] = X1.T @ b2a ; B_p[g,h] = X2.T @ a2b
  out[g] = concat(A_p[g].flat, B_p[g].flat)    -> [G, 2D]

Sharding: data-parallel over graphs. 8 cores x 2 graphs each; weights
replicated. Each core processes its 16 (g,h) pairs as 4 "stacks" of 4 pairs:
partition dim = (pair-in-stack, k) = (4, 32) = 128, free dim = (i', j') = 16384.

Engine allocation (per stack, all sized to hide under the ~14us ACT tanh):
  DVE   : broadcast-multiply at 2x packed mode; all b2a j-fold levels (2x
          contiguous-halves adds); a2b residual reduce; reciprocal.
  ACT   : tanh; softmax exp with fused accum_out sum; small PSUM->SBUF copies.
  PE    : a2b 32-chunk qdiag accumulate; b2a final qdiag matmul; transposes;
          projections.
  GPSIMD: compute-free (its tensor ops contend with DVE for the shared SBUF
          port, slowing both ~4x); used only as a DMA issue queue.
"""
import sys

sys.path.insert(0, "/opt/trn_rl_repo")

from contextlib import ExitStack

import numpy as np

import concourse.bass as bass
import concourse.bacc as bacc
import concourse.mybir as mybir
import concourse.tile as tile
from concourse.masks import make_identity

F32 = mybir.dt.float32
BF16 = mybir.dt.bfloat16

D = 256
H = 8
HD = 32
G = 16
NA = 128
NB = 128
NCORES = 8
GSH = G // NCORES          # graphs per core = 2
NPAIR = GSH * H            # 16 (g,h) pairs per core
SPP = 4                    # pairs per stack
NSTACK = NPAIR // SPP      # 4
NK = HD                    # 32
NJ = 128                   # nodes per head-view
FF = NJ * NJ               # 16384 free elems per stack


def build_kernel():
    nc = bacc.Bacc()
    a_d = nc.dram_tensor("A", [GSH * NA, D], F32, kind="ExternalInput")
    b_d = nc.dram_tensor("B", [GSH * NB, D], F32, kind="ExternalInput")
    w1_d = nc.dram_tensor("W1", [D, D], F32, kind="ExternalInput")
    w2_d = nc.dram_tensor("W2", [D, D], F32, kind="ExternalInput")
    b1_d = nc.dram_tensor("bias1", [1, D], F32, kind="ExternalInput")
    b2_d = nc.dram_tensor("bias2", [1, D], F32, kind="ExternalInput")
    q_d = nc.dram_tensor("q", [1, NK], F32, kind="ExternalInput")
    out_d = nc.dram_tensor("out", [GSH, 2 * D], F32, kind="ExternalOutput")


    with tile.TileContext(nc) as tc, ExitStack() as ctx:
        cst = ctx.enter_context(tc.tile_pool(name="cst", bufs=1))
        sbin = ctx.enter_context(tc.tile_pool(name="sbin", bufs=1))
        sbt = ctx.enter_context(tc.tile_pool(name="sbt", bufs=1))
        sbtr = ctx.enter_context(tc.tile_pool(name="sbtr", bufs=2))
        big = ctx.enter_context(tc.tile_pool(name="big", bufs=3))
        bigp = ctx.enter_context(tc.tile_pool(name="bigp", bufs=3))
        fold1 = ctx.enter_context(tc.tile_pool(name="fold1", bufs=2))
        fold2 = ctx.enter_context(tc.tile_pool(name="fold2", bufs=2))
        fold2b = ctx.enter_context(tc.tile_pool(name="fold2b", bufs=1))
        sm = ctx.enter_context(tc.tile_pool(name="sm", bufs=2))
        pst = ctx.enter_context(tc.tile_pool(name="pst", bufs=2, space="PSUM"))
        psx = ctx.enter_context(tc.tile_pool(name="psx", bufs=2, space="PSUM"))
        pss = ctx.enter_context(tc.tile_pool(name="pss", bufs=2, space="PSUM"))
        ps1 = ctx.enter_context(tc.tile_pool(name="ps1", bufs=1, space="PSUM"))

        ident = cst.tile([128, 128], F32)
        make_identity(nc, ident[:])
        ones1 = cst.tile([1, 128], F32)
        nc.vector.memset(ones1[:], 1.0)
        onescol = cst.tile([128, 1], F32)
        nc.vector.memset(onescol[:], 1.0)

        # ---- load inputs; transpose W1,W2 fully, A/B per row-block ----
        _ldq = [nc.sync, nc.scalar]

        def trans_rowblock(src_d, t_sb, r, name, qi=[0]):
            """transpose rows [128r, 128r+128) of src_d into t_sb[c][:, 128r:+128]"""
            blk = sbin.tile([128, D], F32, tag=f"{name}ld{r}")
            _ldq[qi[0] % 2].dma_start(blk[:], src_d[r * 128:(r + 1) * 128, :])
            qi[0] += 1
            for c in range(2):
                tp = pst.tile([128, 128], F32, tag="tr")
                nc.tensor.transpose(tp[:], blk[:, c * 128:(c + 1) * 128], ident[:])
                nc.scalar.copy(t_sb[c][:, r * 128:(r + 1) * 128], tp[:])

        def alloc_t(name):
            return [sbin.tile([128, D], F32, tag=f"{name}T{c}", name=f"{name}T{c}") for c in range(2)]

        at, bt, w1t, w2t = alloc_t("A"), alloc_t("B"), alloc_t("W1"), alloc_t("W2")
        b1_sb = sbin.tile([1, D], F32)
        b2_sb = sbin.tile([1, D], F32)
        # g0-critical loads first, spread across queues
        trans_rowblock(b_d, bt, 0, "B")
        trans_rowblock(w2_d, w2t, 0, "W2")
        trans_rowblock(w2_d, w2t, 1, "W2")
        trans_rowblock(a_d, at, 0, "A")
        trans_rowblock(w1_d, w1t, 0, "W1")
        trans_rowblock(w1_d, w1t, 1, "W1")
        nc.sync.dma_start(b2_sb[:], b2_d[:])
        nc.scalar.dma_start(b1_sb[:], b1_d[:])

        xs_t = {}

        def xmm(xt, wt, bb, nm, g):
            xp = psx.tile([128, D], F32, tag="xmm")
            nc.tensor.matmul(xp[:], xt[0][:, g * 128:(g + 1) * 128], wt[0][:], start=True, stop=False)
            nc.tensor.matmul(xp[:], xt[1][:, g * 128:(g + 1) * 128], wt[1][:], start=False, stop=False)
            nc.tensor.matmul(xp[:], ones1[0:1, :], bb[:], start=False, stop=True)
            xs = sbin.tile([128, D], F32, tag=f"xs_{nm}{g}", name=f"xs_{nm}{g}")
            nc.scalar.copy(xs[:], xp[:])
            xs_t[(nm, g)] = xs

        x1f, x2f, x1t, x2t = [None] * NSTACK, [None] * NSTACK, [None] * NSTACK, [None] * NSTACK

        def gather_stack(s):
            g, hq = s // 2, s % 2
            for (fl, tl, nm, quad) in ((x1f, x1t, "x1", True), (x2f, x2t, "x2", False)):
                xf = sbt.tile([128, 128], F32, tag=f"{nm}f{s}")
                # SBUF->SBUF permuting gather straight from the xmm output
                # (no DRAM round trip); 4 DMAs to stay within the 3-dim
                # balanced-AP limit. The memset only appeases CoreSim's
                # write tracking, which can't see partition-permuted writes.
                nc.vector.memset(xf[:], 0.0)
                xs = xs_t[(nm, g)]
                srcv = xs[:].rearrange("(hq pp n) (ss k) -> hq n ss pp k",
                                       hq=2, pp=SPP, n=16, ss=H)
                if s < 2:
                    # latency-critical first stacks: spread the 4 transfers
                    # over two queues each so they don't serialize
                    qs = ([nc.gpsimd, nc.scalar] if nm == "x1" else
                          [nc.sync, nc.scalar])
                else:
                    qs = [nc.gpsimd] if nm == "x1" else [nc.sync]
                for pp in range(SPP):
                    qs[pp % len(qs)].dma_start(xf[:, pp * NK:(pp + 1) * NK],
                                               srcv[hq][:, :, pp:pp + 1])
                fl[s] = xf
                tp = pst.tile([128, 128], F32, tag="tr")
                nc.tensor.transpose(tp[:], xf[:], ident[:])
                if quad:
                    xtb = sbtr.tile([128, 4 * 128], BF16, tag=f"{nm}tq")
                    nc.scalar.copy(xtb[:].rearrange("p (i q) -> p i q", q=4),
                                   tp[:].unsqueeze(2).broadcast_to([128, 128, 4]))
                else:
                    xtb = sbtr.tile([128, 128], BF16, tag=f"{nm}tp")
                    nc.scalar.copy(xtb[:], tp[:])
                tl[s] = xtb

        # ---- qdiag [128, SPP]: qdiag[(pp,k), pp'] = q[k] * (pp == pp') ----
        q_sb = cst.tile([1, NK], F32)
        nc.gpsimd.dma_start(q_sb[:], q_d[:])
        q_bf = cst.tile([1, NK], BF16)
        nc.vector.tensor_copy(q_bf[:], q_sb[:])
        qdiag = cst.tile([128, SPP], BF16)
        nc.vector.memset(qdiag[:], 0.0)
        for pp in range(SPP):
            nc.gpsimd.dma_start(qdiag[pp * NK:(pp + 1) * NK, pp:pp + 1], q_bf[:])

        # graph-0 chain first so stack 0 starts ASAP (x2 leads: it trails otherwise)
        xmm(bt, w2t, b2_sb, "x2", 0)
        xmm(at, w1t, b1_sb, "x1", 0)
        gather_stack(0)
        gather_stack(1)
        trans_rowblock(a_d, at, 1, "A")
        trans_rowblock(b_d, bt, 1, "B")
        xmm(bt, w2t, b2_sb, "x2", 1)
        xmm(at, w1t, b1_sb, "x1", 1)
        gather_stack(2)
        gather_stack(3)

        # ---- main loop over stacks, software-pipelined three deep:
        # main(s) -> epi_a(s) after main(s+1) -> epi_b(s) after main(s+2),
        # so per-stack epilogue latency never stalls the in-order engine
        # queues that feed the next stack's multiply/tanh stream.
        probs_at = sm.tile([128, NPAIR], F32, tag="pta")
        probs_bt = sm.tile([128, NPAIR], F32, tag="ptb")
        ADD = mybir.AluOpType.add
        a2b_ps_t, fs1_t = [None] * NSTACK, [None] * NSTACK

        def main_part(s):
            # t4 lives as two half-stack tiles (i 0:64 / 64:128) so the next
            # stack's tanh only back-pressures on the earliest-consumed half.
            t4h = [big.tile([128, FF // 2], BF16, tag="t4", name=f"t4_{s}_{h}")
                   for h in range(2)]
            fs1 = fold1.tile([128, 8192], BF16, tag="fs1")
            fs1_t[s] = fs1
            if s == 0:
                widths_i = (16, 16, 32, 64)
            elif s == NSTACK - 1:
                widths_i = (32, 32, 32, 32)
            else:
                widths_i = (64, 64)

            f1 = fs1[:].rearrange("p (i j) -> p i j", j=64)

            def t4v(i0, i1):
                """view [p, i0:i1, j] across the half-tiles (i0, i1 within one half)"""
                h, ib = i0 // 64, i0 % 64
                return t4h[h][:, ib * NJ:(ib + (i1 - i0)) * NJ]

            # b2a level-1 j-fold (128->64 within each i-row), issued per
            # completed tanh i-range: [p, i, 0:64] + [p, i, 64:128].
            def fold_l1(eng, i0, i1):
                tv = t4v(i0, i1).rearrange("p (i hj j) -> p i hj j", hj=2, j=64)
                eng.tensor_tensor(f1[:, i0:i1],
                                  tv[:, :, 0:1].squeeze(2),
                                  tv[:, :, 1:2].squeeze(2), op=ADD)

            io = 0
            for HH in widths_i:
                p4 = bigp.tile([128, FF // 2], BF16, tag="p4")
                in0 = x1t[s][:, io * 4:(io + HH) * 4].rearrange("p (i q) -> p i q", q=4)\
                    .unsqueeze(2).broadcast_to([128, HH, NJ // 4, 4])
                in1 = x2t[s][:].rearrange("p (j2 q) -> p j2 q", q=4)\
                    .unsqueeze(1).broadcast_to([128, HH, NJ // 4, 4])
                nc.vector.tensor_tensor(
                    p4[:, 0:HH * NJ].rearrange("p (i j2 q) -> p i j2 q", q=4, j2=NJ // 4),
                    in0, in1, op=mybir.AluOpType.mult)
                nc.scalar.activation(t4v(io, io + HH), p4[:, 0:HH * NJ],
                                     mybir.ActivationFunctionType.Tanh)
                io += HH
                if s == NSTACK - 1:
                    fold_l1(nc.vector, io - HH, io)   # tail stack: per tanh chunk
                elif io == 64:
                    fold_l1(nc.vector, 0, 64)
                elif io == 128:
                    fold_l1(nc.vector, 64, 128)

            # a2b: accumulate i'-quads on PE with q-block-diag lhsT
            a2b_ps = pss.tile([SPP, 4 * NJ], F32, tag="a2b")
            nch = FF // (4 * NJ)
            for ch in range(nch):
                half = t4h[ch // (nch // 2)]
                co = (ch % (nch // 2)) * 4 * NJ
                nc.tensor.matmul(a2b_ps[:], qdiag[:], half[:, co:co + 4 * NJ],
                                 start=(ch == 0), stop=(ch == nch - 1))
            a2b_ps_t[s] = a2b_ps

        fsl2_t = [None] * NSTACK

        def softmax_probs(s, lg, pt, nm):
            ex = sm.tile([SPP, NJ], F32, tag=f"ex{nm}")
            sme = sm.tile([SPP, 1], F32, tag=f"sm{nm}")
            nc.scalar.activation(ex[:], lg[:], mybir.ActivationFunctionType.Exp,
                                 scale=1.0 / NJ, accum_out=sme[:])
            rcp = sm.tile([SPP, 1], F32, tag=f"rc{nm}")
            nc.vector.reciprocal(rcp[:], sme[:])
            pr = sm.tile([SPP, NJ], F32, tag=f"pr{nm}")
            nc.scalar.mul(pr[:], ex[:], rcp[:])
            pp_ps = pst.tile([128, SPP], F32, tag="tr")
            nc.tensor.transpose(pp_ps[:], pr[:], ident[0:SPP, 0:SPP])
            nc.scalar.copy(pt[:, s * SPP:(s + 1) * SPP], pp_ps[:])

        _oq = [nc.sync, nc.gpsimd, nc.scalar]

        def project(s, xf, pt, half):
            g, hq = s // 2, s % 2
            pj = ps1.tile([NK, SPP], F32, tag="proj")
            for pp in range(SPP):
                nc.tensor.matmul(pj[:, pp:pp + 1], xf[:, pp * NK:(pp + 1) * NK],
                                 pt[:, s * SPP + pp:s * SPP + pp + 1], start=True, stop=True)
            pjs = sm.tile([NK, SPP], F32, tag=f"projs{s}_{half}")
            nc.scalar.copy(pjs[:], pj[:])
            dst = out_d[g:g + 1, half * D + hq * SPP * HD: half * D + (hq + 1) * SPP * HD]
            dst = dst.rearrange("o (pp k) -> o k pp", pp=SPP)
            _oq[(s * 2 + half) % 3].dma_start(dst, pjs[:])

        def epi_a(s):
            # a2b residual reduce + softmax + x2 projection; b2a fold level 2.
            a2b_t = sm.tile([SPP, NJ], F32, tag="a2bt")
            nc.vector.tensor_reduce(
                a2b_t[:], a2b_ps_t[s][:].rearrange("p (iq j) -> p j iq", iq=4),
                axis=mybir.AxisListType.X, op=mybir.AluOpType.add)
            softmax_probs(s, a2b_t, probs_bt, "b")
            project(s, x2f[s], probs_bt, 1)

            fsl2 = fold2.tile([128, 4096], BF16, tag="fsl2")
            fsl2_t[s] = fsl2
            vin = fs1_t[s][:].rearrange("p (i hj j) -> p i hj j", hj=2, j=32)
            nc.vector.tensor_tensor(fsl2[:].rearrange("p (i j) -> p i j", j=32),
                                    vin[:, :, 0:1].squeeze(2), vin[:, :, 1:2].squeeze(2), op=ADD)

        def epi_b(s):
            # b2a: finish the j-fold on DVE (levels 3..7), qdiag mm, softmax,
            # x1 projection.
            fs = fold2b.tile([128, 3968], BF16, tag="fs2b")
            src, win, off = fsl2_t[s], 32, 0
            base_in = 0
            while win > 1:
                w = win // 2
                vin = src[:, base_in:base_in + 128 * win].rearrange(
                    "p (i hj j) -> p i hj j", hj=2, j=w)
                nc.vector.tensor_tensor(
                    fs[:, off:off + 128 * w].rearrange("p (i j) -> p i j", j=w),
                    vin[:, :, 0:1].squeeze(2), vin[:, :, 1:2].squeeze(2), op=ADD)
                src, base_in, off, win = fs, off, off + 128 * w, w
            b2a_ps = ps1.tile([SPP, NJ], F32, tag="b2a")
            nc.tensor.matmul(b2a_ps[:], qdiag[:], fs[:, 3840:3968], start=True, stop=True)
            softmax_probs(s, b2a_ps, probs_at, "a")
            project(s, x1f[s], probs_at, 0)

        main_part(0)
        main_part(1)
        epi_a(0)
        main_part(2)
        epi_a(1)
        epi_b(0)
        main_part(3)
        epi_a(2)
        epi_b(1)
        epi_a(3)
        epi_b(3)
        epi_b(2)

    if not nc.is_finalized():
        nc.finalize()
    return nc


def shard_inputs(inputs):
    """Full inputs -> list of 8 per-core input maps."""
    A = np.asarray(inputs["A"], np.float32)
    B = np.asarray(inputs["B"], np.float32)
    maps = []
    for c in range(NCORES):
        maps.append({
            "A": np.ascontiguousarray(A[c * GSH * NA:(c + 1) * GSH * NA]),
            "B": np.ascontiguousarray(B[c * GSH * NB:(c + 1) * GSH * NB]),
            "W1": np.asarray(inputs["W1"], np.float32),
            "W2": np.asarray(inputs["W2"], np.float32),
            "bias1": np.asarray(inputs["bias1"], np.float32).reshape(1, D),
            "bias2": np.asarray(inputs["bias2"], np.float32).reshape(1, D),
            "q": np.asarray(inputs["q"], np.float32).reshape(1, NK),
        })
    return maps


_NC_CACHE = {}


def kernel(**inputs) -> np.ndarray:
    """Full (unsharded) inputs -> full [G, 2D] output, running on 8 cores."""
    from concourse.bass_utils import run_bass_kernel_spmd

    if "nc" not in _NC_CACHE:
        _NC_CACHE["nc"] = build_kernel()
    nc = _NC_CACHE["nc"]
    in_maps = shard_inputs(inputs)
    res = run_bass_kernel_spmd(nc, in_maps, core_ids=list(range(NCORES)))
    out = np.concatenate([res.results[c]["out"] for c in range(NCORES)], axis=0)
    return out.astype(np.float32)


if __name__ == "__main__":
    # CoreSim single-core debug: core 0 vs numpy reference
    from concourse.bass_interp import CoreSim

    rng = np.random.default_rng(0)
    scale = 1.0 / np.sqrt(D)
    full = {
        "A": rng.standard_normal((G * NA, D)).astype(np.float32),
        "B": rng.standard_normal((G * NB, D)).astype(np.float32),
        "W1": (rng.standard_normal((D, D)) * scale).astype(np.float32),
        "bias1": (rng.standard_normal(D) * scale).astype(np.float32),
        "W2": (rng.standard_normal((D, D)) * scale).astype(np.float32),
        "bias2": (rng.standard_normal(D) * scale).astype(np.float32),
        "q": (rng.standard_normal(HD) * scale).astype(np.float32),
    }

    def ref_core(m):
        x1 = m["A"] @ m["W1"].T + m["bias1"][0]
        x2 = m["B"] @ m["W2"].T + m["bias2"][0]
        x1 = x1.reshape(GSH, H, NA, HD)
        x2 = x2.reshape(GSH, H, NB, HD)
        att = np.einsum("ghijk,k->ghij", np.tanh(x1[:, :, :, None, :] * x2[:, :, None, :, :]), m["q"][0])

        def smax(v, ax):
            v = v - v.max(axis=ax, keepdims=True)
            e = np.exp(v)
            return e / e.sum(axis=ax, keepdims=True)

        b2a = smax(att.mean(axis=3), 2)
        a2b = smax(att.mean(axis=2), 2)
        A_p = np.einsum("ghik,ghi->ghk", x1, b2a).reshape(GSH, D)
        B_p = np.einsum("ghjk,ghj->ghk", x2, a2b).reshape(GSH, D)
        return np.concatenate([A_p, B_p], axis=1)

    nc = build_kernel()
    m0 = shard_inputs(full)[0]
    sim = CoreSim(nc)
    for k, v in m0.items():
        sim.tensor(k)[:] = v
    sim.simulate()
    got = sim.tensor("out").copy()
    want = ref_core(m0)
    err = np.abs(got - want).max() / np.abs(want).max()
    print("sim time:", sim.time, "ns")
    print("rel err:", err)

